# revision 11
# baseline (speedup 1.0000x reference)
"""DepthWiseIIRConv Trainium2 kernel (8 NeuronCores, channel-sharded).

Math: the module is, per (batch, channel), a LINEAR map on the flattened
16x16 grid: depthwise 3x3 conv (zero pad SAME) followed by the 2D causal
IIR out[h,w] = out[h-1,w-1] + out[h-1,w] + out[h,w-1] + ax[h,w], + bias.

The IIR's transfer coefficients are Delannoy numbers D(dh,dw); composing
with the conv gives a per-channel 256x256 matrix M_c (block-Toeplitz with
top/left boundary corrections), precomputed on host in float64:

    out_flat[b, c, :] = x_flat[b, c, :] @ M_c  + bias_c

Per core (64 channels x 256 batches), variant "v3" (default):
  - x is staged on host into transposed hw-major bf16 layout, so the
    device needs no transposes: per channel two accumulating matmuls
    xT.T @ M_c (K=256 over two 128-partition passes), then a fused
    bias-add + PSUM->SBUF copy (VectorE/ScalarE alternating), DMA out
    as bf16 (upcast on host). ~24 MiB HBM traffic per core.

Other variants kept for A/B: "v3hl" (x split into bf16 hi+lo on host,
4 matmul passes, ~32 MiB), "bf16" (on-chip PE transposes + hi/lo split),
"f32r" (all-fp32 path, ~48 MiB, ~1e-4 rel err).
"""

import numpy as np

H = W = 16
HW = H * W
B = 256
C = 512
NCORES = 8
CPC = C // NCORES  # channels per core = 64
P = 128

VARIANT = "v3"


# ---------------------------------------------------------------- host math
def _delannoy(n: int = H) -> np.ndarray:
    D = np.zeros((n, n), dtype=np.float64)
    D[0, 0] = 1.0
    for i in range(n):
        for j in range(n):
            if i == 0 and j == 0:
                continue
            s = 0.0
            if i > 0:
                s += D[i - 1, j]
            if j > 0:
                s += D[i, j - 1]
            if i > 0 and j > 0:
                s += D[i - 1, j - 1]
            D[i, j] = s
    return D


def build_M(w_a: np.ndarray) -> np.ndarray:
    """w_a: (1, C, 3, 3) -> M: (C, 256, 256) float64 with
    out_flat[b, c, :] = x_flat[b, c, :] @ M[c]  (bias separate)."""
    Cc = w_a.shape[1]
    D = _delannoy()
    Dpad = np.zeros((H + 4, W + 4), dtype=np.float64)  # index m+2 for m in [-2, 17]
    Dpad[2:H + 2, 2:W + 2] = D
    # Dstack[t=(dh,dw), p+1, q+1] = D[p + dh - 1, q + dw - 1]
    Dstack = np.zeros((9, 17, 17), dtype=np.float64)
    for dh in range(3):
        for dw in range(3):
            Dstack[dh * 3 + dw] = Dpad[dh:dh + 17, dw:dw + 17]
    wf = w_a.reshape(Cc, 9).astype(np.float64)
    # Boundary variants: input row h_i=0 cannot receive the dh=2 tap (it
    # would come from conv position h'=-1, killed by zero-pad); same for
    # w_i=0 / dw=2.
    Vmask = np.ones((4, 9), dtype=np.float64)
    for t in range(9):
        dh, dw = divmod(t, 3)
        if dh == 2:
            Vmask[1, t] = 0.0
            Vmask[3, t] = 0.0
        if dw == 2:
            Vmask[2, t] = 0.0
            Vmask[3, t] = 0.0
    Kvar = np.einsum('ct,vt,tpq->cvpq', wf, Vmask, Dstack)  # (C, 4, 17, 17)

    hi = np.repeat(np.arange(H), W)
    wi = np.tile(np.arange(W), H)
    rowtype = (hi == 0) * 1 + (wi == 0) * 2
    dHij = hi[None, :] - hi[:, None]
    dWij = wi[None, :] - wi[:, None]
    valid = (dHij >= -1) & (dWij >= -1)
    dHc = np.clip(dHij + 1, 0, 16)
    dWc = np.clip(dWij + 1, 0, 16)
    M = Kvar[:, rowtype[:, None], dHc, dWc] * valid[None, :, :]
    return M


# ---------------------------------------------------------------- device kernel
_CACHED = {}


def _build_nc_v3(reps: int = 1, hilo: bool = False, GC: int = 16, SC: int = 16,
                 xbufs: int = 2, mbufs: int = 2, obufs: int = 3, pbufs: int = 6):
    """Host-transposed bf16 x; no on-chip transposes."""
    import concourse.bacc as bacc
    import concourse.mybir as mybir
    from concourse import tile

    f32 = mybir.dt.float32
    bf16 = mybir.dt.bfloat16
    NG = CPC // GC
    NXT = 2 if hilo else 1

    nc = bacc.Bacc("TRN2", target_bir_lowering=False, num_devices=NCORES)
    # xt[p, s, kk, c, b] = split s of x[b, c0+c, kk*128+p]  (s: hi[, lo])
    xt_d = nc.dram_tensor("xt", [P, NXT, 2, CPC, B], bf16, kind="ExternalInput")
    m_d = nc.dram_tensor("m", [P, CPC, 2, HW], bf16, kind="ExternalInput")
    bb_d = nc.dram_tensor("biasb", [P, CPC], f32, kind="ExternalInput")
    o_d = nc.dram_tensor("out", [B, CPC, HW], bf16, kind="ExternalOutput")

    with tile.TileContext(nc) as tc:
        with (
            tc.tile_pool(name="const", bufs=1) as constp,
            tc.tile_pool(name="xin", bufs=xbufs) as xp,
            tc.tile_pool(name="mp", bufs=mbufs) as mp,
            tc.tile_pool(name="outs", bufs=obufs) as op,
            tc.tile_pool(name="o_ps", bufs=pbufs, space="PSUM") as opp,
        ):
            biasb = constp.tile([P, CPC], f32)
            nc.sync.dma_start(biasb[:], bb_d[:])

            def _rep_body(_i=None):
                GH = SC if SC else GC // 2
                NH = GC // GH
                for g in range(NG):
                    gs = slice(g * GC, (g + 1) * GC)
                    msb = mp.tile([P, GC, 2, HW], bf16)
                    xt = xp.tile([P, NXT, 2, GC, B], bf16)
                    # half-chunk loads: compute on channels 0..GH-1 starts
                    # as soon as the first halves land
                    for hf in range(NH):
                        hs = slice(g * GC + hf * GH, g * GC + (hf + 1) * GH)
                        ls = slice(hf * GH, (hf + 1) * GH)
                        nc.sync.dma_start(msb[:, ls], m_d[:, hs])
                        nc.sync.dma_start(xt[:, :, :, ls], xt_d[:, :, :, hs])
                    for bh in range(2):
                        bs = slice(bh * P, (bh + 1) * P)
                        ot = op.tile([P, GC, HW], bf16)
                        for ci in range(GC):
                            c = g * GC + ci
                            ops_t = opp.tile([P, HW], f32)
                            n_mm = NXT * 2
                            idx = 0
                            for s in range(NXT):
                                for kk in range(2):
                                    nc.tensor.matmul(
                                        ops_t[:],
                                        xt[:, s, kk, ci, bs],
                                        msb[:, ci, kk],
                                        start=(idx == 0),
                                        stop=(idx == n_mm - 1),
                                    )
                                    idx += 1
                            if ci % 2 == 0:
                                nc.vector.tensor_scalar_add(
                                    ot[:, ci], ops_t[:], biasb[:, c:c + 1]
                                )
                            else:
                                nc.scalar.activation(
                                    ot[:, ci], ops_t[:],
                                    mybir.ActivationFunctionType.Identity,
                                    bias=biasb[:, c:c + 1],
                                )
                        nc.sync.dma_start(o_d[bs, gs], ot[:])

            if reps == 1:
                _rep_body()
            else:
                with tc.For_i(0, reps, 1) as i:
                    _rep_body(i)

    nc.compile()
    return nc


def _build_nc(reps: int = 1, variant: str = VARIANT):
    if variant in ("v3", "v3hl"):
        return _build_nc_v3(reps, hilo=(variant == "v3hl"))
    import concourse.bacc as bacc
    import concourse.mybir as mybir
    from concourse import tile

    f32 = mybir.dt.float32
    f32r = mybir.dt.float32r
    bf16 = mybir.dt.bfloat16
    use_bf = variant == "bf16"
    mdt = bf16 if use_bf else f32r
    odt = bf16 if use_bf else f32
    GC = 8
    NG = CPC // GC

    nc = bacc.Bacc("TRN2", target_bir_lowering=False, num_devices=NCORES)
    x_d = nc.dram_tensor("x", [B, CPC, HW], f32, kind="ExternalInput")
    m_d = nc.dram_tensor("m", [P, CPC, 2, HW], mdt, kind="ExternalInput")
    bb_d = nc.dram_tensor("biasb", [P, CPC], f32, kind="ExternalInput")
    id_d = nc.dram_tensor("ident", [P, P], f32, kind="ExternalInput")
    o_d = nc.dram_tensor("out", [B, CPC, HW], odt, kind="ExternalOutput")

    with tile.TileContext(nc) as tc:
        with (
            tc.tile_pool(name="const", bufs=1) as constp,
            tc.tile_pool(name="xin", bufs=3) as xp,
            tc.tile_pool(name="xT", bufs=3) as xtp,
            tc.tile_pool(name="outs", bufs=3) as op,
            tc.tile_pool(name="tp_ps", bufs=4, space="PSUM") as tpp,
            tc.tile_pool(name="o_ps", bufs=4, space="PSUM") as opp,
        ):
            ident = constp.tile([P, P], f32)
            nc.sync.dma_start(ident[:], id_d[:])
            biasb = constp.tile([P, CPC], f32)
            nc.sync.dma_start(biasb[:], bb_d[:])
            msb = constp.tile([P, CPC, 2, HW], mdt)  # resident (8 or 16 MiB)

            for rep in range(reps):
                for bh in range(2):
                    for g in range(NG):
                        if bh == 0:  # reload per rep so marginal rep == full kernel
                            nc.sync.dma_start(
                                msb[:, g * GC:(g + 1) * GC],
                                m_d[:, g * GC:(g + 1) * GC],
                            )
                        xt = xp.tile([P, GC, HW], f32)
                        nc.sync.dma_start(
                            xt[:], x_d[bh * P:(bh + 1) * P, g * GC:(g + 1) * GC]
                        )
                        ot = op.tile([P, GC, HW], odt)
                        for ci in range(GC):
                            c = g * GC + ci
                            xTh = xtp.tile([P, HW], mdt, tag="xTh")
                            if use_bf:
                                xTl = xtp.tile([P, HW], bf16, tag="xTl")
                            for kk in range(2):
                                tp = tpp.tile([P, P], f32)
                                nc.tensor.transpose(
                                    tp[:], xt[:, ci, kk * P:(kk + 1) * P], ident[:]
                                )
                                sl = slice(kk * P, (kk + 1) * P)
                                if kk == 0:
                                    nc.vector.tensor_copy(xTh[:, sl], tp[:])
                                else:
                                    nc.scalar.copy(xTh[:, sl], tp[:])
                                if use_bf:
                                    nc.vector.tensor_sub(
                                        xTl[:, sl], tp[:], xTh[:, sl]
                                    )
                            ops_t = opp.tile([P, HW], f32)
                            if use_bf:
                                parts = [(xTh, 0), (xTh, 1), (xTl, 0), (xTl, 1)]
                            else:
                                parts = [(xTh, 0), (xTh, 1)]
                            for idx, (src, kk) in enumerate(parts):
                                nc.tensor.matmul(
                                    ops_t[:],
                                    src[:, kk * P:(kk + 1) * P],
                                    msb[:, c, kk],
                                    start=(idx == 0),
                                    stop=(idx == len(parts) - 1),
                                )
                            nc.vector.tensor_scalar_add(
                                ot[:, ci], ops_t[:], biasb[:, c:c + 1]
                            )
                        nc.sync.dma_start(
                            o_d[bh * P:(bh + 1) * P, g * GC:(g + 1) * GC], ot[:]
                        )

    nc.compile()
    return nc


def _get_nc(reps: int = 1, variant: str = VARIANT):
    key = f"nc-{variant}-{reps}"
    if key not in _CACHED:
        _CACHED[key] = _build_nc(reps, variant)
    return _CACHED[key]


def _make_in_maps(x, w_a, bias, variant: str = VARIANT):
    import ml_dtypes

    bf = ml_dtypes.bfloat16
    M = build_M(w_a)  # float64
    M = M.astype(bf) if variant != "f32r" else M.astype(np.float32)

    in_maps = []
    for i in range(NCORES):
        c0 = i * CPC
        Mc = np.asarray(M[c0:c0 + CPC]).reshape(CPC, 2, P, HW)
        m_core = np.ascontiguousarray(np.transpose(Mc, (2, 0, 1, 3)))  # (P,CPC,2,HW)
        bb = np.ascontiguousarray(
            np.broadcast_to(bias.reshape(C)[c0:c0 + CPC], (P, CPC))
        )
        if variant in ("v3", "v3hl"):
            xc = x[:, c0:c0 + CPC].reshape(B, CPC, 2, P)
            # xt[p, kk, c, b] = x[b, c, kk*128+p]
            xtr = np.ascontiguousarray(np.transpose(xc, (3, 2, 1, 0)))  # (P,2,CPC,B)
            xh = xtr.astype(bf)
            if variant == "v3hl":
                xl = (xtr - xh.astype(np.float32)).astype(bf)
                xt = np.ascontiguousarray(
                    np.stack([xh, xl], axis=1)
                )  # (P, 2, 2, CPC, B)
            else:
                xt = np.ascontiguousarray(xh[:, None])  # (P, 1, 2, CPC, B)
            in_maps.append({"xt": xt, "m": m_core, "biasb": bb})
        else:
            xc = np.ascontiguousarray(x[:, c0:c0 + CPC].reshape(B, CPC, HW))
            ident = np.eye(P, dtype=np.float32)
            in_maps.append({"x": xc, "m": m_core, "biasb": bb, "ident": ident})
    return in_maps


def kernel(x: np.ndarray, w_a: np.ndarray, bias: np.ndarray, trace: bool = False,
           reps: int = 1, variant: str = VARIANT):
    from concourse.bass_utils import run_bass_kernel_spmd

    x = np.ascontiguousarray(np.asarray(x, dtype=np.float32))
    w_a = np.asarray(w_a, dtype=np.float32)
    bias = np.asarray(bias, dtype=np.float32)

    in_maps = _make_in_maps(x, w_a, bias, variant)
    nc = _get_nc(reps, variant)
    res = run_bass_kernel_spmd(nc, in_maps, core_ids=list(range(NCORES)), trace=trace)

    out = np.empty((B, C, H, W), dtype=np.float32)
    for i in range(NCORES):
        c0 = i * CPC
        out[:, c0:c0 + CPC] = res.results[i]["out"].astype(np.float32).reshape(
            B, CPC, H, W
        )
    if trace:
        return out, res
    return out


# revision 12
# speedup vs baseline: 1.1836x; 1.1836x over previous
"""DepthWiseIIRConv Trainium2 kernel (8 NeuronCores, channel-sharded).

Math: the module is, per (batch, channel), a LINEAR map on the flattened
16x16 grid: depthwise 3x3 conv (zero pad SAME) followed by the 2D causal
IIR out[h,w] = out[h-1,w-1] + out[h-1,w] + out[h,w-1] + ax[h,w], + bias.

The IIR's transfer coefficients are Delannoy numbers D(dh,dw); composing
with the conv gives a per-channel 256x256 matrix M_c (block-Toeplitz with
top/left boundary corrections), precomputed on host in float64:

    out_flat[b, c, :] = x_flat[b, c, :] @ M_c  + bias_c

Per core (64 channels x 256 batches), variant "v3" (default):
  - x is staged on host into transposed hw-major bf16 layout, so the
    device needs no transposes: per channel two accumulating matmuls
    xT.T @ M_c (K=256 over two 128-partition passes), then a fused
    bias-add + PSUM->SBUF copy (VectorE/ScalarE alternating), DMA out
    as bf16 (upcast on host). ~24 MiB HBM traffic per core.

Other variants kept for A/B: "v3hl" (x split into bf16 hi+lo on host,
4 matmul passes, ~32 MiB), "bf16" (on-chip PE transposes + hi/lo split),
"f32r" (all-fp32 path, ~48 MiB, ~1e-4 rel err).
"""

import numpy as np

H = W = 16
HW = H * W
B = 256
C = 512
NCORES = 8
CPC = C // NCORES  # channels per core = 64
P = 128

VARIANT = "v3"


# ---------------------------------------------------------------- host math
def _delannoy(n: int = H) -> np.ndarray:
    D = np.zeros((n, n), dtype=np.float64)
    D[0, 0] = 1.0
    for i in range(n):
        for j in range(n):
            if i == 0 and j == 0:
                continue
            s = 0.0
            if i > 0:
                s += D[i - 1, j]
            if j > 0:
                s += D[i, j - 1]
            if i > 0 and j > 0:
                s += D[i - 1, j - 1]
            D[i, j] = s
    return D


def build_M(w_a: np.ndarray) -> np.ndarray:
    """w_a: (1, C, 3, 3) -> M: (C, 256, 256) float64 with
    out_flat[b, c, :] = x_flat[b, c, :] @ M[c]  (bias separate)."""
    Cc = w_a.shape[1]
    D = _delannoy()
    Dpad = np.zeros((H + 4, W + 4), dtype=np.float64)  # index m+2 for m in [-2, 17]
    Dpad[2:H + 2, 2:W + 2] = D
    # Dstack[t=(dh,dw), p+1, q+1] = D[p + dh - 1, q + dw - 1]
    Dstack = np.zeros((9, 17, 17), dtype=np.float64)
    for dh in range(3):
        for dw in range(3):
            Dstack[dh * 3 + dw] = Dpad[dh:dh + 17, dw:dw + 17]
    wf = w_a.reshape(Cc, 9).astype(np.float64)
    # Boundary variants: input row h_i=0 cannot receive the dh=2 tap (it
    # would come from conv position h'=-1, killed by zero-pad); same for
    # w_i=0 / dw=2.
    Vmask = np.ones((4, 9), dtype=np.float64)
    for t in range(9):
        dh, dw = divmod(t, 3)
        if dh == 2:
            Vmask[1, t] = 0.0
            Vmask[3, t] = 0.0
        if dw == 2:
            Vmask[2, t] = 0.0
            Vmask[3, t] = 0.0
    Kvar = np.einsum('ct,vt,tpq->cvpq', wf, Vmask, Dstack)  # (C, 4, 17, 17)

    hi = np.repeat(np.arange(H), W)
    wi = np.tile(np.arange(W), H)
    rowtype = (hi == 0) * 1 + (wi == 0) * 2
    dHij = hi[None, :] - hi[:, None]
    dWij = wi[None, :] - wi[:, None]
    valid = (dHij >= -1) & (dWij >= -1)
    dHc = np.clip(dHij + 1, 0, 16)
    dWc = np.clip(dWij + 1, 0, 16)
    M = Kvar[:, rowtype[:, None], dHc, dWc] * valid[None, :, :]
    return M


# ---------------------------------------------------------------- device kernel
_CACHED = {}


def _build_nc_v3(reps: int = 1, hilo: bool = False, GC: int = 16, SC: int = 16,
                 xbufs: int = 2, mbufs: int = 2, obufs: int = 3, pbufs: int = 6):
    """Host-transposed bf16 x; no on-chip transposes."""
    import concourse.bacc as bacc
    import concourse.mybir as mybir
    from concourse import tile

    f32 = mybir.dt.float32
    bf16 = mybir.dt.bfloat16
    NG = CPC // GC
    NXT = 2 if hilo else 1

    # Packed M: column-group A (j<112, h_j<7) only needs contraction rows
    # k<128 (h_k<=7); the k>=128 block there is structurally zero and is not
    # shipped. Free-dim pack per channel: [A(128,112) | B0(128,144) | B1(128,144)].
    MF = 112 + 144 + 144  # 400 packed columns per channel
    nc = bacc.Bacc("TRN2", target_bir_lowering=False, num_devices=NCORES)
    # xt[p, s, kk, c, b] = split s of x[b, c0+c, kk*128+p]  (s: hi[, lo])
    xt_d = nc.dram_tensor("xt", [P, NXT, 2, CPC, B], bf16, kind="ExternalInput")
    m_d = nc.dram_tensor("m", [P, CPC, MF], bf16, kind="ExternalInput")
    bb_d = nc.dram_tensor("biasb", [P, CPC], f32, kind="ExternalInput")
    o_d = nc.dram_tensor("out", [B, CPC, HW], bf16, kind="ExternalOutput")

    with tile.TileContext(nc) as tc:
        with (
            tc.tile_pool(name="const", bufs=1) as constp,
            tc.tile_pool(name="xin", bufs=xbufs) as xp,
            tc.tile_pool(name="mp", bufs=mbufs) as mp,
            tc.tile_pool(name="outs", bufs=obufs) as op,
            tc.tile_pool(name="o_ps", bufs=pbufs, space="PSUM") as opp,
        ):
            biasb = constp.tile([P, CPC], f32)
            nc.sync.dma_start(biasb[:], bb_d[:])

            def _rep_body(_i=None):
                GH = SC if SC else GC // 2
                NH = GC // GH
                for g in range(NG):
                    gs = slice(g * GC, (g + 1) * GC)
                    msb = mp.tile([P, GC, MF], bf16)
                    xt = xp.tile([P, NXT, 2, GC, B], bf16)
                    # half-chunk loads: compute on channels 0..GH-1 starts
                    # as soon as the first halves land
                    for hf in range(NH):
                        hs = slice(g * GC + hf * GH, g * GC + (hf + 1) * GH)
                        ls = slice(hf * GH, (hf + 1) * GH)
                        nc.sync.dma_start(msb[:, ls], m_d[:, hs])
                        nc.sync.dma_start(xt[:, :, :, ls], xt_d[:, :, :, hs])
                    for bh in range(2):
                        bs = slice(bh * P, (bh + 1) * P)
                        ot = op.tile([P, GC, HW], bf16)
                        for ci in range(GC):
                            c = g * GC + ci
                            ops_t = opp.tile([P, HW], f32)
                            for s in range(NXT):
                                st = s == 0
                                sp = s == NXT - 1
                                # A: out cols 0..111, K = kk0 only
                                nc.tensor.matmul(
                                    ops_t[:, 0:112],
                                    xt[:, s, 0, ci, bs],
                                    msb[:, ci, 0:112],
                                    start=st, stop=sp,
                                )
                                # B: out cols 112..255, K = kk0 + kk1
                                nc.tensor.matmul(
                                    ops_t[:, 112:256],
                                    xt[:, s, 0, ci, bs],
                                    msb[:, ci, 112:256],
                                    start=st, stop=False,
                                )
                                nc.tensor.matmul(
                                    ops_t[:, 112:256],
                                    xt[:, s, 1, ci, bs],
                                    msb[:, ci, 256:400],
                                    start=False, stop=sp,
                                )
                            if ci % 2 == 0:
                                nc.vector.tensor_scalar_add(
                                    ot[:, ci], ops_t[:], biasb[:, c:c + 1]
                                )
                            else:
                                nc.scalar.activation(
                                    ot[:, ci], ops_t[:],
                                    mybir.ActivationFunctionType.Identity,
                                    bias=biasb[:, c:c + 1],
                                )
                        nc.sync.dma_start(o_d[bs, gs], ot[:])

            if reps == 1:
                _rep_body()
            else:
                with tc.For_i(0, reps, 1) as i:
                    _rep_body(i)

    nc.compile()
    return nc


def _build_nc(reps: int = 1, variant: str = VARIANT):
    if variant in ("v3", "v3hl"):
        return _build_nc_v3(reps, hilo=(variant == "v3hl"))
    import concourse.bacc as bacc
    import concourse.mybir as mybir
    from concourse import tile

    f32 = mybir.dt.float32
    f32r = mybir.dt.float32r
    bf16 = mybir.dt.bfloat16
    use_bf = variant == "bf16"
    mdt = bf16 if use_bf else f32r
    odt = bf16 if use_bf else f32
    GC = 8
    NG = CPC // GC

    nc = bacc.Bacc("TRN2", target_bir_lowering=False, num_devices=NCORES)
    x_d = nc.dram_tensor("x", [B, CPC, HW], f32, kind="ExternalInput")
    m_d = nc.dram_tensor("m", [P, CPC, 2, HW], mdt, kind="ExternalInput")
    bb_d = nc.dram_tensor("biasb", [P, CPC], f32, kind="ExternalInput")
    id_d = nc.dram_tensor("ident", [P, P], f32, kind="ExternalInput")
    o_d = nc.dram_tensor("out", [B, CPC, HW], odt, kind="ExternalOutput")

    with tile.TileContext(nc) as tc:
        with (
            tc.tile_pool(name="const", bufs=1) as constp,
            tc.tile_pool(name="xin", bufs=3) as xp,
            tc.tile_pool(name="xT", bufs=3) as xtp,
            tc.tile_pool(name="outs", bufs=3) as op,
            tc.tile_pool(name="tp_ps", bufs=4, space="PSUM") as tpp,
            tc.tile_pool(name="o_ps", bufs=4, space="PSUM") as opp,
        ):
            ident = constp.tile([P, P], f32)
            nc.sync.dma_start(ident[:], id_d[:])
            biasb = constp.tile([P, CPC], f32)
            nc.sync.dma_start(biasb[:], bb_d[:])
            msb = constp.tile([P, CPC, 2, HW], mdt)  # resident (8 or 16 MiB)

            for rep in range(reps):
                for bh in range(2):
                    for g in range(NG):
                        if bh == 0:  # reload per rep so marginal rep == full kernel
                            nc.sync.dma_start(
                                msb[:, g * GC:(g + 1) * GC],
                                m_d[:, g * GC:(g + 1) * GC],
                            )
                        xt = xp.tile([P, GC, HW], f32)
                        nc.sync.dma_start(
                            xt[:], x_d[bh * P:(bh + 1) * P, g * GC:(g + 1) * GC]
                        )
                        ot = op.tile([P, GC, HW], odt)
                        for ci in range(GC):
                            c = g * GC + ci
                            xTh = xtp.tile([P, HW], mdt, tag="xTh")
                            if use_bf:
                                xTl = xtp.tile([P, HW], bf16, tag="xTl")
                            for kk in range(2):
                                tp = tpp.tile([P, P], f32)
                                nc.tensor.transpose(
                                    tp[:], xt[:, ci, kk * P:(kk + 1) * P], ident[:]
                                )
                                sl = slice(kk * P, (kk + 1) * P)
                                if kk == 0:
                                    nc.vector.tensor_copy(xTh[:, sl], tp[:])
                                else:
                                    nc.scalar.copy(xTh[:, sl], tp[:])
                                if use_bf:
                                    nc.vector.tensor_sub(
                                        xTl[:, sl], tp[:], xTh[:, sl]
                                    )
                            ops_t = opp.tile([P, HW], f32)
                            if use_bf:
                                parts = [(xTh, 0), (xTh, 1), (xTl, 0), (xTl, 1)]
                            else:
                                parts = [(xTh, 0), (xTh, 1)]
                            for idx, (src, kk) in enumerate(parts):
                                nc.tensor.matmul(
                                    ops_t[:],
                                    src[:, kk * P:(kk + 1) * P],
                                    msb[:, c, kk],
                                    start=(idx == 0),
                                    stop=(idx == len(parts) - 1),
                                )
                            nc.vector.tensor_scalar_add(
                                ot[:, ci], ops_t[:], biasb[:, c:c + 1]
                            )
                        nc.sync.dma_start(
                            o_d[bh * P:(bh + 1) * P, g * GC:(g + 1) * GC], ot[:]
                        )

    nc.compile()
    return nc


def _get_nc(reps: int = 1, variant: str = VARIANT):
    key = f"nc-{variant}-{reps}"
    if key not in _CACHED:
        _CACHED[key] = _build_nc(reps, variant)
    return _CACHED[key]


def _make_in_maps(x, w_a, bias, variant: str = VARIANT):
    import ml_dtypes

    bf = ml_dtypes.bfloat16
    M = build_M(w_a)  # float64
    M = M.astype(bf) if variant != "f32r" else M.astype(np.float32)

    in_maps = []
    for i in range(NCORES):
        c0 = i * CPC
        Mc = np.asarray(M[c0:c0 + CPC])  # (CPC, 256, 256)
        if variant in ("v3", "v3hl"):
            # pack: [A=(k<128, j<112) | B0=(k<128, j>=112) | B1=(k>=128, j>=112)]
            m_core = np.empty((P, CPC, 400), dtype=Mc.dtype)
            m_core[:, :, 0:112] = np.transpose(Mc[:, 0:128, 0:112], (1, 0, 2))
            m_core[:, :, 112:256] = np.transpose(Mc[:, 0:128, 112:256], (1, 0, 2))
            m_core[:, :, 256:400] = np.transpose(Mc[:, 128:256, 112:256], (1, 0, 2))
            m_core = np.ascontiguousarray(m_core)
        else:
            Mc4 = Mc.reshape(CPC, 2, P, HW)
            m_core = np.ascontiguousarray(np.transpose(Mc4, (2, 0, 1, 3)))
        bb = np.ascontiguousarray(
            np.broadcast_to(bias.reshape(C)[c0:c0 + CPC], (P, CPC))
        )
        if variant in ("v3", "v3hl"):
            xc = x[:, c0:c0 + CPC].reshape(B, CPC, 2, P)
            # xt[p, kk, c, b] = x[b, c, kk*128+p]
            xtr = np.ascontiguousarray(np.transpose(xc, (3, 2, 1, 0)))  # (P,2,CPC,B)
            xh = xtr.astype(bf)
            if variant == "v3hl":
                xl = (xtr - xh.astype(np.float32)).astype(bf)
                xt = np.ascontiguousarray(
                    np.stack([xh, xl], axis=1)
                )  # (P, 2, 2, CPC, B)
            else:
                xt = np.ascontiguousarray(xh[:, None])  # (P, 1, 2, CPC, B)
            in_maps.append({"xt": xt, "m": m_core, "biasb": bb})
        else:
            xc = np.ascontiguousarray(x[:, c0:c0 + CPC].reshape(B, CPC, HW))
            ident = np.eye(P, dtype=np.float32)
            in_maps.append({"x": xc, "m": m_core, "biasb": bb, "ident": ident})
    return in_maps


def kernel(x: np.ndarray, w_a: np.ndarray, bias: np.ndarray, trace: bool = False,
           reps: int = 1, variant: str = VARIANT):
    from concourse.bass_utils import run_bass_kernel_spmd

    x = np.ascontiguousarray(np.asarray(x, dtype=np.float32))
    w_a = np.asarray(w_a, dtype=np.float32)
    bias = np.asarray(bias, dtype=np.float32)

    in_maps = _make_in_maps(x, w_a, bias, variant)
    nc = _get_nc(reps, variant)
    res = run_bass_kernel_spmd(nc, in_maps, core_ids=list(range(NCORES)), trace=trace)

    out = np.empty((B, C, H, W), dtype=np.float32)
    for i in range(NCORES):
        c0 = i * CPC
        out[:, c0:c0 + CPC] = res.results[i]["out"].astype(np.float32).reshape(
            B, CPC, H, W
        )
    if trace:
        return out, res
    return out
